# revision 9
# baseline (speedup 1.0000x reference)
"""Trainium2 Bass kernel for BiDAF-style bidirectional attention.

Reference math (per batch b):
    sim[c,q]  = q[q]·wq + c[c]·wc + sum_e wm[e]*question[q,e]*context[c,e]
    c2q[c,:]  = softmax_q(sim[c,:]) @ question          # (C, E)
    q2c[:]    = softmax_c(max_q sim[c,:]) @ context     # (E,)
    out[c,:]  = [context | c2q | context*c2q | context*q2c]

Sharding: pure data parallel over batch (B=16 -> 2 batches per core x 8 cores).

Layout strategy per 128-row context tile (C on partitions):
  - transpose Xc on the PE array -> XcT (E on partitions)
  - sim psum (128, 129) = ones x qw_aug  (K=1 matmul adds q-weighted row)
                        + XcT0.T @ rhs0 + XcT1.T @ rhs1   (fp32r)
    where rhs[:, j, 0:128] = (wm * question^T) chunk and rhs[:, j, 128] = wc
    chunk, so column 128 of sim = context·wc (needed only for q2c stats).
  - softmax over free dim (Q) with exp+rowsum fused on the scalar engine
  - c2q = attn^T.T @ question (fp32r, N=256), row-rescaled during PSUM->SBUF
  - q2c: two-pass over the 16 tiles using rowmax stats (max over Q incl. wc),
    global max/sum via PE-transpose + ones-matmul broadcasts.
"""

import numpy as np

import concourse.bass as bass
import concourse.tile as tile
import concourse.mybir as mybir
from concourse import bacc
from concourse.bass_utils import run_bass_kernel_spmd
from concourse.masks import make_identity

B, C, Q, E = 16, 2048, 128, 256
NCORES = 8
BPC = B // NCORES          # batches per core
NT = C // 128              # context tiles per batch
F32 = mybir.dt.float32
F32R = mybir.dt.float32r

# fp32r matmuls run at 1 cycle/row for N>=256 (vs 4 for fp32), but operands
# must be *written* as float32r by their producer (BIR verifier enforces the
# rounding). DMA-produced tiles stay fp32; compute-produced feeder tiles are
# written as f32r.
MM_DT = F32R


def _body(tc, out_ext, ctx_in, q_in, wq_in, wc_in, wm_in):
    nc = tc.nc
    with (
        tc.tile_pool(name="singles", bufs=1) as singles,
        tc.tile_pool(name="xcpool", bufs=2 * NT + 2) as xcp,
        tc.tile_pool(name="qside", bufs=2) as qside,
        tc.tile_pool(name="work", bufs=4) as work,
        tc.tile_pool(name="outbuf", bufs=4) as outp,
        tc.tile_pool(name="statsp", bufs=2) as statsp,
        tc.tile_pool(name="ps_xct", bufs=2, space="PSUM") as ps_xct,
        tc.tile_pool(name="ps_sim", bufs=2, space="PSUM") as ps_sim,
        tc.tile_pool(name="ps_pt", bufs=1, space="PSUM") as ps_pt,
        tc.tile_pool(name="ps_c2q", bufs=1, space="PSUM") as ps_c2q,
        tc.tile_pool(name="ps_misc", bufs=2, space="PSUM") as ps_misc,
    ):
        ident = singles.tile([128, 128], F32)
        make_identity(nc, ident)
        ones_r = singles.tile([1, 128], F32)
        nc.vector.memset(ones_r, 1.0)
        ones_c = singles.tile([128, 1], F32)
        nc.vector.memset(ones_c, 1.0)
        # rank-1 params laid out (128 partitions, 2 chunks of E)
        wq_sb = singles.tile([128, 2], F32)
        nc.sync.dma_start(out=wq_sb, in_=wq_in.rearrange("(j p) -> p j", p=128))
        wc_sb = singles.tile([128, 2], F32)
        nc.sync.dma_start(out=wc_sb, in_=wc_in.rearrange("(j p) -> p j", p=128))
        wm_sb = singles.tile([128, 2], F32)
        nc.sync.dma_start(out=wm_sb, in_=wm_in.rearrange("(j p) -> p j", p=128))

        for b in range(BPC):
            # ---- phase A: question-side prep -------------------------------
            qm = qside.tile([128, E], F32)
            nc.sync.dma_start(out=qm, in_=q_in[b])
            qmt_ps = ps_xct.tile([128, E], F32, tag="xct")
            for j in range(2):
                nc.tensor.transpose(
                    qmt_ps[:, j * 128 : (j + 1) * 128],
                    qm[:, j * 128 : (j + 1) * 128],
                    ident,
                )
            qmt_sb = qside.tile([128, E], F32)
            nc.vector.tensor_copy(out=qmt_sb, in_=qmt_ps)
            # rounded copy of the question for the fp32r c2q matmul
            qm_r = qside.tile([128, E], MM_DT)
            nc.vector.tensor_copy(out=qm_r, in_=qm)
            # rhs_aug[:, j, 0:128] = wm-chunk * QmT-chunk ; [:, j, 128] = wc-chunk
            rhs_aug = qside.tile([128, 2, 130], MM_DT)
            for j in range(2):
                nc.vector.tensor_scalar_mul(
                    rhs_aug[:, j, 0:128],
                    qmt_sb[:, j * 128 : (j + 1) * 128],
                    wm_sb[:, j : j + 1],
                )
                nc.vector.tensor_copy(
                    out=rhs_aug[:, j, 128:129], in_=wc_sb[:, j : j + 1]
                )
                # memset can't write f32r; a x0.0 DVE op produces rounded zeros
                nc.vector.tensor_scalar_mul(
                    rhs_aug[:, j, 129:130], wc_sb[:, j : j + 1], 0.0
                )
            # q_weighted row (1, Q) then augmented with 0 for the wc column
            qw_ps = ps_misc.tile([1, 128], F32, tag="misc")
            for j in range(2):
                nc.tensor.matmul(
                    qw_ps,
                    wq_sb[:, j : j + 1],
                    qmt_sb[:, j * 128 : (j + 1) * 128],
                    start=(j == 0),
                    stop=(j == 1),
                )
            qw_aug = qside.tile([1, 130], F32)
            nc.vector.tensor_copy(out=qw_aug[:, 0:128], in_=qw_ps)
            nc.vector.memset(qw_aug[:, 128:130], 0.0)

            mstat = statsp.tile([128, NT], F32)
            xcs = []
            recips = []

            # ---- phase B: pass 1 over context tiles ------------------------
            for t in range(NT):
                xc = xcp.tile([128, E], F32, tag="xc")
                xcs.append(xc)
                nc.sync.dma_start(
                    out=xc, in_=ctx_in[b, t * 128 : (t + 1) * 128, :]
                )
                xct_ps = ps_xct.tile([128, E], F32, tag="xct")
                for j in range(2):
                    nc.tensor.transpose(
                        xct_ps[:, j * 128 : (j + 1) * 128],
                        xc[:, j * 128 : (j + 1) * 128],
                        ident,
                    )
                xct_sb = work.tile([128, E], MM_DT, tag="xct_sb")
                nc.vector.tensor_copy(out=xct_sb, in_=xct_ps)

                sim_ps = ps_sim.tile([128, 130], F32, tag="sim")
                nc.tensor.matmul(sim_ps, ones_r, qw_aug, start=True, stop=False)
                for j in range(2):
                    nc.tensor.matmul(
                        sim_ps,
                        xct_sb[:, j * 128 : (j + 1) * 128],
                        rhs_aug[:, j, :],
                        start=False,
                        stop=(j == 1),
                    )

                neg_m = work.tile([128, 1], F32, tag="neg_m")
                nc.vector.reduce_max(
                    out=neg_m,
                    in_=sim_ps[:, 0:128],
                    axis=mybir.AxisListType.X,
                    negate=True,
                )
                # q2c stats: max over q of full sim = wc·ctx (col 128) + rowmax
                nc.vector.tensor_sub(mstat[:, t : t + 1], sim_ps[:, 128:129], neg_m)

                p_sb = work.tile([128, 128], F32, tag="p_sb")
                row_sum = work.tile([128, 1], F32, tag="row_sum")
                nc.scalar.activation(
                    out=p_sb,
                    in_=sim_ps[:, 0:128],
                    func=mybir.ActivationFunctionType.Exp,
                    bias=neg_m,
                    scale=1.0,
                    accum_out=row_sum,
                )
                recip = statsp.tile([128, 1], F32, tag="recip")
                recips.append(recip)
                nc.vector.reciprocal(out=recip, in_=row_sum)

                pt_ps = ps_pt.tile([128, 128], F32, tag="pt")
                nc.tensor.transpose(pt_ps, p_sb, ident)
                pt_sb = work.tile([128, 128], MM_DT, tag="pt_sb")
                nc.vector.tensor_copy(out=pt_sb, in_=pt_ps)

                c2q_ps = ps_c2q.tile([128, E], F32, tag="c2q")
                nc.tensor.matmul(c2q_ps, pt_sb, qm_r, start=True, stop=True)
                c2q_sb = outp.tile([128, E], F32, tag="c2q_sb")
                nc.scalar.activation(
                    out=c2q_sb,
                    in_=c2q_ps,
                    func=mybir.ActivationFunctionType.Copy,
                    scale=recip,
                )
                prod_sb = outp.tile([128, E], F32, tag="prod_sb")
                nc.vector.tensor_mul(prod_sb, xc, c2q_sb)

                rows = slice(t * 128, (t + 1) * 128)
                nc.sync.dma_start(out=out_ext[b, rows, 0:E], in_=xc)
                nc.sync.dma_start(out=out_ext[b, rows, E : 2 * E], in_=c2q_sb)
                nc.sync.dma_start(out=out_ext[b, rows, 2 * E : 3 * E], in_=prod_sb)

            # ---- phase C: q2c attention over the context axis --------------
            r1 = statsp.tile([128, 1], F32, tag="r1")
            nc.vector.reduce_max(
                out=r1, in_=mstat, axis=mybir.AxisListType.X
            )
            r1t_ps = ps_misc.tile([1, 128], F32, tag="misc")
            nc.tensor.transpose(r1t_ps, r1, ident)
            neg_gmax = statsp.tile([1, 1], F32, tag="neg_gmax")
            nc.vector.reduce_max(
                out=neg_gmax, in_=r1t_ps, axis=mybir.AxisListType.X, negate=True
            )
            ngb_ps = ps_misc.tile([128, 1], F32, tag="misc")
            nc.tensor.matmul(ngb_ps, ones_r, neg_gmax, start=True, stop=True)
            ngb_sb = statsp.tile([128, 1], F32, tag="ngb_sb")
            nc.vector.tensor_copy(out=ngb_sb, in_=ngb_ps)

            e_sb = statsp.tile([128, NT], F32, tag="e_sb")
            s_col = statsp.tile([128, 1], F32, tag="s_col")
            nc.scalar.activation(
                out=e_sb,
                in_=mstat,
                func=mybir.ActivationFunctionType.Exp,
                bias=ngb_sb,
                scale=1.0,
                accum_out=s_col,
            )
            tot_ps = ps_misc.tile([1, 1], F32, tag="misc")
            nc.tensor.matmul(tot_ps, s_col, ones_c, start=True, stop=True)
            rt_sb = statsp.tile([1, 1], F32, tag="rt_sb")
            nc.vector.reciprocal(out=rt_sb, in_=tot_ps)

            q2c_ps = ps_misc.tile([1, E], F32, tag="misc")
            for t in range(NT):
                nc.tensor.matmul(
                    q2c_ps,
                    e_sb[:, t : t + 1],
                    xcs[t],
                    start=(t == 0),
                    stop=(t == NT - 1),
                )
            q2c_sb = statsp.tile([1, E], F32, tag="q2c_sb")
            nc.scalar.activation(
                out=q2c_sb,
                in_=q2c_ps,
                func=mybir.ActivationFunctionType.Copy,
                scale=rt_sb,
            )
            q2cb_ps = ps_misc.tile([128, E], F32, tag="misc")
            nc.tensor.matmul(q2cb_ps, ones_r, q2c_sb, start=True, stop=True)
            q2cb_sb = statsp.tile([128, E], F32, tag="q2cb_sb")
            nc.vector.tensor_copy(out=q2cb_sb, in_=q2cb_ps)

            # ---- phase D: pass 2 -> context * q2c ---------------------------
            for t in range(NT):
                out3 = outp.tile([128, E], F32, tag="out3")
                nc.vector.tensor_mul(out3, xcs[t], q2cb_sb)
                rows = slice(t * 128, (t + 1) * 128)
                nc.sync.dma_start(out=out_ext[b, rows, 3 * E : 4 * E], in_=out3)


_NC_CACHE = None


def _build():
    global _NC_CACHE
    if _NC_CACHE is not None:
        return _NC_CACHE
    nc = bacc.Bacc(
        "TRN2", target_bir_lowering=False, debug=False, num_devices=NCORES
    )
    ctx_in = nc.dram_tensor("context", [BPC, C, E], F32, kind="ExternalInput").ap()
    q_in = nc.dram_tensor("question", [BPC, Q, E], F32, kind="ExternalInput").ap()
    wq_in = nc.dram_tensor("w_question", [E], F32, kind="ExternalInput").ap()
    wc_in = nc.dram_tensor("w_context", [E], F32, kind="ExternalInput").ap()
    wm_in = nc.dram_tensor("w_multiple", [E], F32, kind="ExternalInput").ap()
    out_ext = nc.dram_tensor("out", [BPC, C, 4 * E], F32, kind="ExternalOutput").ap()
    with tile.TileContext(nc) as tc:
        _body(tc, out_ext, ctx_in, q_in, wq_in, wc_in, wm_in)
    nc.compile()
    _NC_CACHE = nc
    return nc


def _run(inputs, trace=False, **kw):
    nc = _build()
    context = np.ascontiguousarray(np.asarray(inputs["context"], dtype=np.float32))
    question = np.ascontiguousarray(np.asarray(inputs["question"], dtype=np.float32))
    wq = np.ascontiguousarray(np.asarray(inputs["w_question"], dtype=np.float32))
    wc = np.ascontiguousarray(np.asarray(inputs["w_context"], dtype=np.float32))
    wm = np.ascontiguousarray(np.asarray(inputs["w_multiple"], dtype=np.float32))
    in_maps = []
    for i in range(NCORES):
        sl = slice(i * BPC, (i + 1) * BPC)
        in_maps.append(
            {
                "context": context[sl],
                "question": question[sl],
                "w_question": wq,
                "w_context": wc,
                "w_multiple": wm,
            }
        )
    res = run_bass_kernel_spmd(
        nc, in_maps, core_ids=list(range(NCORES)), trace=trace, **kw
    )
    out = np.concatenate([res.results[i]["out"] for i in range(NCORES)], axis=0)
    return out, res


def kernel(**inputs):
    out, _ = _run(inputs, trace=False)
    return out
